# revision 1
# baseline (speedup 1.0000x reference)
"""Trainium2 Bass kernel for nn_BaselineMNISTClassifier (vq_codebook).

reference:
    x = samples - 0.5                        # [B, F]
    hv = einsum('bf,df->bd', x, bhv)         # [B, D]
    e = (hv > 0)                             # binary
    ham[b, c] = sum_d |e - centroids[c, d]|  # [B, C]
    return -ham

Identity used on device: with e' = (hv > 0) - 0.5 in {-1/2, +1/2} and
cmod = 1 - 2c in {-1, +1}:  |e - c| = e' * cmod + 1/2, so
    ham[b, c] = sum_d e'[b, d] * cmod[c, d] + D/2
which turns the broadcast Hamming into a second (tiny) matmul over the
same d-tiles.

Sharding: the D axis (10000) splits across 8 cores, 1250 (zero-padded
to 1280) per core. Every core sees the full batch and computes a
partial hamming [C, B]; the partials sum on the host (padded dims
contribute exactly 0: the centroid pad value 0.5 makes cmod = 0 there).

The encode matmul runs in float32r (~tf32 precision; streams one
column per cycle at N=512, measured 227 ns / matmul warm). Both
operands are host-transposed so the contraction dim F sits on SBUF
partitions; no on-device transposes anywhere. The hamming matmul runs
in bf16 (e', cmod are exact in bf16), so the device output is exact
integer arithmetic given the encode bits.

Perf structure (per core, measured ~181 us on hardware):
  - 52 warmup matmuls on dummy data release the PE HAM clock gate
    (1.2 -> 2.4 GHz) while the inputs stream in; the clock then stays
    warm for the whole kernel
  - input tiles are single-assignment (no slot reuse), so input DMAs
    never carry data-dependency waits; x triggers issue from SP, w and
    centroid/output triggers from GpSimd (each DMA trigger costs
    ~0.6 us of issue time on its engine)
  - fi-outer / bb-inner matmul order: 4 consecutive matmuls share the
    stationary weights, hiding the fused fp32r LDWEIGHTS
  - all four hamming accumulators of a b-group live in ONE PSUM bank
    at partition offsets 0/32/64/96 via col-tiled matmuls
    (tile_position), which frees 7 PSUM banks for the encode
    accumulation (deep multi-buffering, no start-of-group stalls)
  - hamming matmuls are emitted one d-tile late so the PE never waits
    on the DVE binarize; the epilogue alternates Scalar/DVE and each
    output block DMAs out as soon as its accumulation closes

Toolchain notes: built on bacc.Bacc (its compile() legalizes the
1-sync-wait-per-instruction hardware limit via event semaphores, which
raw Bass + TileContext does not); output DMAs go through nc.gpsimd
because SP DMA_DIRECT2D triggers only take a single wait.
"""

import sys

sys.path.insert(0, "/opt/trn_rl_repo")

import numpy as np

import concourse.bacc as bacc
import concourse.bass as bass
import concourse.mybir as mybir
import concourse.tile as tile
from concourse.bass_utils import run_bass_kernel_spmd

B = 4096
F = 784
D = 10000
C = 10
NCORES = 8
DREAL = D // NCORES          # 1250 real dims per core
DP = 1280                    # padded to 10 d-tiles of 128
ND = DP // 128               # 10
NB = B // 512                # 8 b-blocks of 512
FT = [(i * 128, min(128, F - i * 128)) for i in range((F + 127) // 128)]
NF = len(FT)                 # 7 (6x128 + 16)
NWARM = 52                   # PE warmup matmuls
NFILL = 5                    # dummy matmuls per fi-step of the first group

F32 = mybir.dt.float32
F32R = mybir.dt.float32r
BF16 = mybir.dt.bfloat16
OP = mybir.AluOpType
AF = mybir.ActivationFunctionType

_NC_CACHE = {}


def _build_nc():
    if "nc" in _NC_CACHE:
        return _NC_CACHE["nc"]
    nc = bacc.Bacc("TRN2", debug=False, target_bir_lowering=False)
    xT = nc.dram_tensor("xT", [F, B], F32R, kind="ExternalInput")
    wT = nc.dram_tensor("wT", [F, DP], F32R, kind="ExternalInput")
    cT = nc.dram_tensor("cT", [DP, C], F32, kind="ExternalInput")
    out = nc.dram_tensor("out", [C, B], F32, kind="ExternalOutput")

    with tile.TileContext(nc) as tc:
        with (
            tc.tile_pool(name="dum", bufs=2) as dumpool,
            tc.tile_pool(name="xp", bufs=NB // 2 * NF) as xpool,
            tc.tile_pool(name="wp", bufs=(ND + 1) // 2 * NF) as wpool,
            tc.tile_pool(name="cp", bufs=1) as cpool,
            tc.tile_pool(name="cmp", bufs=1) as cmpool,
            tc.tile_pool(name="ep", bufs=8) as epool,
            tc.tile_pool(name="op", bufs=4) as opool,
            tc.tile_pool(name="pse", bufs=7, space="PSUM") as psepool,
            tc.tile_pool(name="ps2", bufs=1, space="PSUM") as ps2pool,
        ):
            # --- PE warmup: release the HAM clock gate while inputs load.
            wdum = dumpool.tile([128, 128], BF16)
            nc.gpsimd.memset(wdum[:], 1.0)
            xdum = dumpool.tile([128, 512], BF16)
            nc.gpsimd.memset(xdum[:], 1.0)
            psdum = psepool.tile([128, 512], F32, name="psdum", tag="pse")
            for i in range(NWARM):
                nc.tensor.matmul(psdum[:], wdum[:], xdum[:],
                                 start=(i == 0), stop=(i == NWARM - 1))

            # --- centroid prep: one DMA for all 10 d-tiles, then
            # cmod = 1 - 2c (bf16). Pad rows are 0.5 -> cmod = 0.
            ct = cpool.tile([128, ND * C], F32)
            nc.gpsimd.dma_start(
                ct[:].rearrange("p (a c) -> p a c", c=C),
                cT.ap().rearrange("(a p) c -> p a c", p=128))
            cmod = cmpool.tile([128, ND * C], BF16)
            nc.scalar.activation(cmod[:], ct[:], AF.Copy, bias=1.0,
                                 scale=-2.0)
            cmods = [cmod[:, di * C:(di + 1) * C] for di in range(ND)]

            # --- input loads; tiles single-assignment (loaded once, no
            # slot reuse) so input DMAs never carry data waits. x tiles
            # span two b-blocks, w tiles two d-tiles.
            xts = {}
            wts = {}

            def load_x(bp, fi):   # bp = b-block pair index (0..3)
                f0, fl = FT[fi]
                xt = xpool.tile([fl, 1024], F32R, name=f"xt_{bp}_{fi}",
                                tag="xt")
                nc.sync.dma_start(
                    xt[:], xT[f0:f0 + fl, bp * 1024:(bp + 1) * 1024])
                # center (x - 0.5) in place on DVE
                nc.vector.tensor_scalar_add(xt[:], xt[:], -0.5)
                xts[bp, fi] = xt

            def load_w(dp, fi):   # dp = d-tile pair index (0..4)
                f0, fl = FT[fi]
                wid = min(256, DP - dp * 256)
                wt = wpool.tile([fl, wid], F32R, name=f"wt_{dp}_{fi}",
                                tag="wt")
                nc.gpsimd.dma_start(
                    wt[:], wT[f0:f0 + fl, dp * 256:dp * 256 + wid])
                wts[dp, fi] = wt

            for i in range(5):
                for fi in range(NF):
                    if i < 4:
                        load_x(i, fi)
                    load_w(i, fi)

            def xop(bb, fi):
                return xts[bb // 2, fi][:, (bb % 2) * 512:(bb % 2 + 1) * 512]

            def wop(di, fi):
                return wts[di // 2, fi][:, (di % 2) * 128:(di % 2 + 1) * 128]

            # --- main compute: two b-groups of 4 blocks.
            for bg in range(2):
                bbs = list(range(bg * 4, bg * 4 + 4))
                ps2 = ps2pool.tile([128, 512], F32, name=f"ps2_{bg}",
                                   tag="ps2")
                psum2 = {bb: ps2[32 * (bb % 4):32 * (bb % 4) + C, :]
                         for bb in bbs}
                pending = []
                for di in range(ND):
                    pses = {}
                    for bb in bbs:
                        pses[bb] = psepool.tile([128, 512], F32,
                                                name=f"pse_{di % 2}_{bb}",
                                                tag="pse")
                    for fi in range(NF):
                        for bb in bbs:
                            nc.tensor.matmul(pses[bb][:], wop(di, fi),
                                             xop(bb, fi),
                                             start=(fi == 0),
                                             stop=(fi == NF - 1))
                    ets = {}
                    for bb in bbs:
                        # e' = (hv > 0) - 0.5 in {-1/2, +1/2}; the last
                        # d-tile binarizes in halves so its hamming
                        # matmuls overlap the binarize (no encode work
                        # left to hide the chain behind)
                        et = epool.tile([128, 512], BF16,
                                        name=f"et_{di % 2}_{bb}", tag="et")
                        if di == ND - 1:
                            for h in range(2):
                                sl = slice(h * 256, (h + 1) * 256)
                                nc.vector.tensor_scalar(
                                    et[:, sl], pses[bb][:, sl], 0.0, 0.5,
                                    op0=OP.is_gt, op1=OP.subtract)
                        else:
                            nc.vector.tensor_scalar(et[:], pses[bb][:],
                                                    0.0, 0.5,
                                                    op0=OP.is_gt,
                                                    op1=OP.subtract)
                        ets[bb] = et
                    for pdi, pbb, pet in pending:
                        nc.tensor.matmul(psum2[pbb], cmods[pdi],
                                         pet[:], start=(pdi == 0),
                                         stop=(pdi == ND - 1),
                                         tile_position=(0, 32 * (pbb % 4)))
                    pending = [(di, bb, ets[bb]) for bb in bbs]
                for pdi, pbb, pet in pending:
                    for h in range(2):
                        sl = slice(h * 256, (h + 1) * 256)
                        nc.tensor.matmul(psum2[pbb][:, sl], cmods[pdi],
                                         pet[:, sl], start=(pdi == 0),
                                         stop=(pdi == ND - 1),
                                         tile_position=(0, 32 * (pbb % 4)))
                    # out = -(psum2 + DREAL/2); alternate engines so the
                    # four epilogues drain in parallel
                    ot = opool.tile([C, 512], F32, name=f"ot_{pbb % 4}",
                                    tag="ot")
                    if pbb % 2 == 0:
                        nc.scalar.activation(ot[:], psum2[pbb], AF.Copy,
                                             bias=-float(DREAL) / 2.0,
                                             scale=-1.0)
                    else:
                        nc.vector.tensor_scalar(ot[:], psum2[pbb],
                                                float(DREAL) / 2.0, -1.0,
                                                op0=OP.add, op1=OP.mult)
                    nc.gpsimd.dma_start(
                        out[:, pbb * 512:(pbb + 1) * 512], ot[:])
    nc.compile()
    _NC_CACHE["nc"] = nc
    return nc


def _prep_in_maps(samples, bhv_matrix, centroids):
    samples = np.ascontiguousarray(samples, dtype=np.float32)
    bhv_matrix = np.ascontiguousarray(bhv_matrix, dtype=np.float32)
    centroids = np.ascontiguousarray(centroids, dtype=np.float32)
    xT = np.ascontiguousarray(samples.T)  # [F, B]
    in_maps = []
    for k in range(NCORES):
        lo_, hi_ = k * DREAL, (k + 1) * DREAL
        wTk = np.zeros((F, DP), dtype=np.float32)
        wTk[:, :DREAL] = bhv_matrix[lo_:hi_, :].T
        cTk = np.full((DP, C), 0.5, dtype=np.float32)
        cTk[:DREAL, :] = centroids[:, lo_:hi_].T
        in_maps.append({"xT": xT, "wT": wTk, "cT": cTk})
    return in_maps


def _run(samples, bhv_matrix, centroids, **spmd_kwargs):
    nc = _build_nc()
    in_maps = _prep_in_maps(samples, bhv_matrix, centroids)
    res = run_bass_kernel_spmd(nc, in_maps, core_ids=list(range(NCORES)),
                               **spmd_kwargs)
    acc = np.zeros((C, B), dtype=np.float32)
    for r in res.results:
        acc += r["out"]
    return np.ascontiguousarray(acc.T), res


def kernel(samples, bhv_matrix, centroids):
    out, _ = _run(samples, bhv_matrix, centroids)
    return out



# revision 3
# speedup vs baseline: 1.5929x; 1.5929x over previous
"""Trainium2 Bass kernel for nn_BaselineMNISTClassifier (vq_codebook).

reference:
    x = samples - 0.5                        # [B, F]
    hv = einsum('bf,df->bd', x, bhv)         # [B, D]
    e = (hv > 0)                             # binary
    ham[b, c] = sum_d |e - centroids[c, d]|  # [B, C]
    return -ham

Identity used on device: with e' = (hv > 0) - 0.5 in {-1/2, +1/2} and
cmod = 1 - 2c in {-1, +1}:  |e - c| = e' * cmod + 1/2, so
    ham[b, c] = sum_d e'[b, d] * cmod[c, d] + D/2
which turns the broadcast Hamming into a second (tiny) matmul over the
same d-tiles.

Sharding: the D axis (10000) splits across 8 cores, 1250 (zero-padded
to 1280) per core. Every core sees the full batch and computes a
partial hamming [C, B]; the partials sum on the host (padded dims
contribute exactly 0: the centroid pad value 0.5 makes cmod = 0 there).

Encode matmul runs in fp8 e4m3 with perf_mode=DoubleRow: 784 features
= 3 DoubleRow pairs (k=2x128) + one plain k=16 fp8 tail per output
tile, vs 7 fp32r matmuls before (~1.6x less PE stream time; measured
rel err 8e-3 vs the 2e-2 gate, validated in numpy against the fp32
reference). x is centered and cast to e4m3 on the host, which also
removes the on-device DVE centering pass. The hamming matmul stays
bf16 (e', cmod exact in bf16) with all four accumulators of a b-group
col-tiled into ONE PSUM bank (tile_position), freeing 7 banks for the
encode accumulation (DoubleRow + col tiling are mutually exclusive on
the XBUS budget, so the hamming matmuls must stay off perf_mode).

Perf structure (per core):
  - 52 warmup matmuls on dummy data release the PE HAM clock gate
    (1.2 -> 2.4 GHz) while the inputs stream in
  - input tiles are single-assignment (no slot reuse), so input DMAs
    never carry data-dependency waits; x triggers issue from SP, w and
    centroid/output triggers from GpSimd
  - j-outer / bb-inner matmul order: 4 consecutive matmuls share the
    stationary weights, hiding the DoubleRow LDWEIGHTS
  - hamming matmuls are emitted one d-tile late so the PE never waits
    on the DVE binarize; the epilogue alternates Scalar/DVE and each
    output block DMAs out as soon as its accumulation closes

Toolchain notes: built on bacc.Bacc (its compile() legalizes the
1-sync-wait-per-instruction hardware limit via event semaphores);
output DMAs go through nc.gpsimd because SP DMA_DIRECT2D triggers
only take a single wait.
"""

import sys

sys.path.insert(0, "/opt/trn_rl_repo")

import ml_dtypes
import numpy as np

import concourse.bacc as bacc
import concourse.bass as bass
import concourse.mybir as mybir
import concourse.tile as tile
from concourse.bass_utils import run_bass_kernel_spmd

B = 4096
F = 784
D = 10000
C = 10
NCORES = 8
DREAL = D // NCORES          # 1250 real dims per core
DP = 1280                    # padded to 10 d-tiles of 128
ND = DP // 128               # 10
NB = B // 512                # 8 b-blocks of 512
NJ = 3                       # DoubleRow f-pairs (k=256 each)
FTAIL = F - NJ * 256         # 16 leftover features
NWARM = 52                   # PE warmup matmuls

F32 = mybir.dt.float32
FP8 = mybir.dt.float8e4
BF16 = mybir.dt.bfloat16
OP = mybir.AluOpType
AF = mybir.ActivationFunctionType
DR = mybir.MatmulPerfMode.DoubleRow

NP_FP8 = mybir.dt.np(FP8)

_NC_CACHE = {}


def _build_nc():
    if "nc" in _NC_CACHE:
        return _NC_CACHE["nc"]
    nc = bacc.Bacc("TRN2", debug=False, target_bir_lowering=False)
    xT = nc.dram_tensor("xT", [F, B], FP8, kind="ExternalInput")
    wT = nc.dram_tensor("wT", [F, DP], FP8, kind="ExternalInput")
    cT = nc.dram_tensor("cT", [DP, C], F32, kind="ExternalInput")
    out = nc.dram_tensor("out", [C, B], F32, kind="ExternalOutput")

    with tile.TileContext(nc) as tc:
        with (
            tc.tile_pool(name="dum", bufs=2) as dumpool,
            tc.tile_pool(name="xp", bufs=NB // 2 * NJ) as xpool,
            tc.tile_pool(name="xtp", bufs=NB // 2) as xtpool,
            tc.tile_pool(name="wp", bufs=(ND + 1) // 2 * NJ) as wpool,
            tc.tile_pool(name="wtp", bufs=(ND + 1) // 2) as wtpool,
            tc.tile_pool(name="cp", bufs=1) as cpool,
            tc.tile_pool(name="cmp", bufs=1) as cmpool,
            tc.tile_pool(name="ep", bufs=8) as epool,
            tc.tile_pool(name="op", bufs=4) as opool,
            tc.tile_pool(name="pse", bufs=7, space="PSUM") as psepool,
            tc.tile_pool(name="ps2", bufs=1, space="PSUM") as ps2pool,
        ):
            # --- PE warmup: release the HAM clock gate while inputs load.
            wdum = dumpool.tile([128, 128], BF16)
            nc.gpsimd.memset(wdum[:], 1.0)
            xdum = dumpool.tile([128, 512], BF16)
            nc.gpsimd.memset(xdum[:], 1.0)
            psdum = psepool.tile([128, 512], F32, name="psdum", tag="pse")
            for i in range(NWARM):
                nc.tensor.matmul(psdum[:], wdum[:], xdum[:],
                                 start=(i == 0), stop=(i == NWARM - 1))

            # --- centroid prep: one DMA for all 10 d-tiles, then
            # cmod = 1 - 2c (bf16). Pad rows are 0.5 -> cmod = 0.
            ct = cpool.tile([128, ND * C], F32)
            nc.gpsimd.dma_start(
                ct[:].rearrange("p (a c) -> p a c", c=C),
                cT.ap().rearrange("(a p) c -> p a c", p=128))
            cmod = cmpool.tile([128, ND * C], BF16)
            nc.scalar.activation(cmod[:], ct[:], AF.Copy, bias=1.0,
                                 scale=-2.0)
            cmods = [cmod[:, di * C:(di + 1) * C] for di in range(ND)]

            # --- input loads; tiles single-assignment (loaded once, no
            # slot reuse) so input DMAs never carry data waits. x tiles
            # span two b-blocks, w tiles two d-tiles. DoubleRow pairs
            # (k-tile t in dim 1) are stacked per-tile: element
            # [p, t, n] is feature 256j + 128t + p.
            xts = {}
            xtails = {}
            wts = {}
            wtails = {}

            def load_x(bp, j):    # bp = b-block pair index (0..3)
                xt = xpool.tile([128, 2, 1024], FP8, name=f"xt_{bp}_{j}",
                                tag="xt")
                nc.sync.dma_start(
                    xt[:],
                    xT[256 * j:256 * (j + 1),
                       bp * 1024:(bp + 1) * 1024].rearrange(
                           "(t p) b -> p t b", p=128))
                xts[bp, j] = xt

            def load_xtail(bp):
                xt = xtpool.tile([FTAIL, 1024], FP8, name=f"xtl_{bp}",
                                 tag="xtl")
                nc.sync.dma_start(
                    xt[:], xT[NJ * 256:F, bp * 1024:(bp + 1) * 1024])
                xtails[bp] = xt

            def load_w(dp, j):    # dp = d-tile pair index (0..4)
                wid = min(256, DP - dp * 256)
                wt = wpool.tile([128, 2, wid], FP8, name=f"wt_{dp}_{j}",
                                tag="wt")
                nc.gpsimd.dma_start(
                    wt[:],
                    wT[256 * j:256 * (j + 1),
                       dp * 256:dp * 256 + wid].rearrange(
                           "(t p) d -> p t d", p=128))
                wts[dp, j] = wt

            def load_wtail(dp):
                wid = min(256, DP - dp * 256)
                wt = wtpool.tile([FTAIL, wid], FP8, name=f"wtl_{dp}",
                                 tag="wtl")
                nc.gpsimd.dma_start(
                    wt[:], wT[NJ * 256:F, dp * 256:dp * 256 + wid])
                wtails[dp] = wt

            for i in range(5):
                for j in range(NJ):
                    if i < 4:
                        load_x(i, j)
                    load_w(i, j)
                if i < 4:
                    load_xtail(i)
                load_wtail(i)

            def xop(bb, j):
                return xts[bb // 2, j][:, :,
                                       (bb % 2) * 512:(bb % 2 + 1) * 512]

            def xtailop(bb):
                return xtails[bb // 2][:, (bb % 2) * 512:(bb % 2 + 1) * 512]

            def wop(di, j):
                return wts[di // 2, j][:, :,
                                       (di % 2) * 128:(di % 2 + 1) * 128]

            def wtailop(di):
                return wtails[di // 2][:, (di % 2) * 128:(di % 2 + 1) * 128]

            # --- main compute: two b-groups of 4 blocks.
            for bg in range(2):
                bbs = list(range(bg * 4, bg * 4 + 4))
                ps2 = ps2pool.tile([128, 512], F32, name=f"ps2_{bg}",
                                   tag="ps2")
                psum2 = {bb: ps2[32 * (bb % 4):32 * (bb % 4) + C, :]
                         for bb in bbs}
                pending = []
                for di in range(ND):
                    pses = {}
                    for bb in bbs:
                        pses[bb] = psepool.tile([128, 512], F32,
                                                name=f"pse_{di % 2}_{bb}",
                                                tag="pse")
                    for j in range(NJ):
                        for bb in bbs:
                            nc.tensor.matmul(pses[bb][:], wop(di, j),
                                             xop(bb, j),
                                             start=(j == 0), stop=False,
                                             perf_mode=DR)
                    for bb in bbs:
                        nc.tensor.matmul(pses[bb][:], wtailop(di),
                                         xtailop(bb),
                                         start=False, stop=True)
                    ets = {}
                    for bb in bbs:
                        # e' = (hv > 0) - 0.5 in {-1/2, +1/2}; the last
                        # d-tile binarizes in halves so its hamming
                        # matmuls overlap the binarize (no encode work
                        # left to hide the chain behind)
                        et = epool.tile([128, 512], BF16,
                                        name=f"et_{di % 2}_{bb}", tag="et")
                        if di == ND - 1:
                            for h in range(2):
                                sl = slice(h * 256, (h + 1) * 256)
                                nc.vector.tensor_scalar(
                                    et[:, sl], pses[bb][:, sl], 0.0, 0.5,
                                    op0=OP.is_gt, op1=OP.subtract)
                        else:
                            nc.vector.tensor_scalar(et[:], pses[bb][:],
                                                    0.0, 0.5,
                                                    op0=OP.is_gt,
                                                    op1=OP.subtract)
                        ets[bb] = et
                    for pdi, pbb, pet in pending:
                        nc.tensor.matmul(psum2[pbb], cmods[pdi],
                                         pet[:], start=(pdi == 0),
                                         stop=(pdi == ND - 1),
                                         tile_position=(0, 32 * (pbb % 4)))
                    pending = [(di, bb, ets[bb]) for bb in bbs]
                for pdi, pbb, pet in pending:
                    for h in range(2):
                        sl = slice(h * 256, (h + 1) * 256)
                        nc.tensor.matmul(psum2[pbb][:, sl], cmods[pdi],
                                         pet[:, sl], start=(pdi == 0),
                                         stop=(pdi == ND - 1),
                                         tile_position=(0, 32 * (pbb % 4)))
                    # out = -(psum2 + DREAL/2); alternate engines so the
                    # four epilogues drain in parallel
                    ot = opool.tile([C, 512], F32, name=f"ot_{pbb % 4}",
                                    tag="ot")
                    if pbb % 2 == 0:
                        nc.scalar.activation(ot[:], psum2[pbb], AF.Copy,
                                             bias=-float(DREAL) / 2.0,
                                             scale=-1.0)
                    else:
                        nc.vector.tensor_scalar(ot[:], psum2[pbb],
                                                float(DREAL) / 2.0, -1.0,
                                                op0=OP.add, op1=OP.mult)
                    nc.gpsimd.dma_start(
                        out[:, pbb * 512:(pbb + 1) * 512], ot[:])
    nc.compile()
    _NC_CACHE["nc"] = nc
    return nc


def _prep_in_maps(samples, bhv_matrix, centroids):
    samples = np.ascontiguousarray(samples, dtype=np.float32)
    bhv_matrix = np.ascontiguousarray(bhv_matrix, dtype=np.float32)
    centroids = np.ascontiguousarray(centroids, dtype=np.float32)
    xT = np.ascontiguousarray((samples.T - 0.5).astype(NP_FP8))  # [F, B]
    w8 = bhv_matrix.astype(NP_FP8)
    in_maps = []
    for k in range(NCORES):
        lo_, hi_ = k * DREAL, (k + 1) * DREAL
        wTk = np.zeros((F, DP), dtype=NP_FP8)
        wTk[:, :DREAL] = w8[lo_:hi_, :].T
        cTk = np.full((DP, C), 0.5, dtype=np.float32)
        cTk[:DREAL, :] = centroids[:, lo_:hi_].T
        in_maps.append({"xT": xT, "wT": wTk, "cT": cTk})
    return in_maps


def _run(samples, bhv_matrix, centroids, **spmd_kwargs):
    nc = _build_nc()
    in_maps = _prep_in_maps(samples, bhv_matrix, centroids)
    res = run_bass_kernel_spmd(nc, in_maps, core_ids=list(range(NCORES)),
                               **spmd_kwargs)
    acc = np.zeros((C, B), dtype=np.float32)
    for r in res.results:
        acc += r["out"]
    return np.ascontiguousarray(acc.T), res


def kernel(samples, bhv_matrix, centroids):
    out, _ = _run(samples, bhv_matrix, centroids)
    return out


# revision 6
# speedup vs baseline: 1.6524x; 1.0373x over previous
"""Trainium2 Bass kernel for nn_BaselineMNISTClassifier (vq_codebook).

reference:
    x = samples - 0.5                        # [B, F]
    hv = einsum('bf,df->bd', x, bhv)         # [B, D]
    e = (hv > 0)                             # binary
    ham[b, c] = sum_d |e - centroids[c, d]|  # [B, C]
    return -ham

Identity used on device: with e' = (hv > 0) - 0.5 in {-1/2, +1/2} and
cmod = 1 - 2c in {-1, +1}:  |e - c| = e' * cmod + 1/2, so
    ham[b, c] = sum_d e'[b, d] * cmod[c, d] + D/2
which turns the broadcast Hamming into a second (tiny) matmul over the
same d-tiles.

Sharding: the D axis (10000) splits across 8 cores, 1250 (zero-padded
to 1280) per core. Every core sees the full batch and computes a
partial hamming [C, B]; the partials sum on the host (padded dims
contribute exactly 0: the centroid pad value 0.5 makes cmod = 0 there).

Encode matmul runs in fp8 e4m3 with perf_mode=DoubleRow: 784 features
= 3 DoubleRow pairs (k=2x128) + one plain k=16 fp8 tail per output
tile, vs 7 fp32r matmuls before (~1.6x less PE stream time; measured
rel err 8e-3 vs the 2e-2 gate, validated in numpy against the fp32
reference). x is centered and cast to e4m3 on the host, which also
removes the on-device DVE centering pass. The hamming matmul stays
bf16 (e', cmod exact in bf16) with all four accumulators of a b-group
col-tiled into ONE PSUM bank (tile_position), freeing 7 banks for the
encode accumulation (DoubleRow + col tiling are mutually exclusive on
the XBUS budget, so the hamming matmuls must stay off perf_mode).

Perf structure (per core):
  - 52 warmup matmuls on dummy data release the PE HAM clock gate
    (1.2 -> 2.4 GHz) while the inputs stream in
  - input tiles are single-assignment (no slot reuse), so input DMAs
    never carry data-dependency waits; x triggers issue from SP, w and
    centroid/output triggers from GpSimd
  - j-outer / bb-inner matmul order: 4 consecutive matmuls share the
    stationary weights, hiding the DoubleRow LDWEIGHTS
  - hamming matmuls are emitted one d-tile late so the PE never waits
    on the DVE binarize; the epilogue alternates Scalar/DVE and each
    output block DMAs out as soon as its accumulation closes

Toolchain notes: built on bacc.Bacc (its compile() legalizes the
1-sync-wait-per-instruction hardware limit via event semaphores);
output DMAs go through nc.gpsimd because SP DMA_DIRECT2D triggers
only take a single wait.
"""

import sys

sys.path.insert(0, "/opt/trn_rl_repo")

import ml_dtypes
import numpy as np

import concourse.bacc as bacc
import concourse.bass as bass
import concourse.mybir as mybir
import concourse.tile as tile
from concourse.bass_utils import run_bass_kernel_spmd

B = 4096
F = 784
D = 10000
C = 10
NCORES = 8
DREAL = D // NCORES          # 1250 real dims per core
DP = 1280                    # padded to 10 d-tiles of 128
ND = DP // 128               # 10
NB = B // 512                # 8 b-blocks of 512
NJ = 3                       # DoubleRow f-pairs (k=256 each)
FTAIL = F - NJ * 256         # 16 leftover features
NWARM = 16                   # PE warmup matmuls (sized to cover the
                             # input-DMA wait; the HAM clock ramp
                             # finishes during the first real d-tiles)

F32 = mybir.dt.float32
FP8 = mybir.dt.float8e4
BF16 = mybir.dt.bfloat16
OP = mybir.AluOpType
AF = mybir.ActivationFunctionType
DR = mybir.MatmulPerfMode.DoubleRow

NP_FP8 = mybir.dt.np(FP8)

_NC_CACHE = {}


def _build_nc():
    if "nc" in _NC_CACHE:
        return _NC_CACHE["nc"]
    nc = bacc.Bacc("TRN2", debug=False, target_bir_lowering=False)
    xT = nc.dram_tensor("xT", [F, B], FP8, kind="ExternalInput")
    wT = nc.dram_tensor("wT", [F, DP], FP8, kind="ExternalInput")
    cT = nc.dram_tensor("cT", [DP, C], F32, kind="ExternalInput")
    out = nc.dram_tensor("out", [C, B], F32, kind="ExternalOutput")

    with tile.TileContext(nc) as tc:
        with (
            tc.tile_pool(name="dum", bufs=2) as dumpool,
            tc.tile_pool(name="xp", bufs=NB * NJ) as xpool,
            tc.tile_pool(name="xtp", bufs=NB // 2) as xtpool,
            tc.tile_pool(name="wp", bufs=(ND + 1) // 2 * NJ) as wpool,
            tc.tile_pool(name="wtp", bufs=(ND + 1) // 2) as wtpool,
            tc.tile_pool(name="cp", bufs=1) as cpool,
            tc.tile_pool(name="cmp", bufs=1) as cmpool,
            tc.tile_pool(name="ep", bufs=8) as epool,
            tc.tile_pool(name="op", bufs=4) as opool,
            tc.tile_pool(name="pse", bufs=7, space="PSUM") as psepool,
            tc.tile_pool(name="ps2", bufs=1, space="PSUM") as ps2pool,
        ):
            # --- PE warmup: release the HAM clock gate while inputs load.
            wdum = dumpool.tile([128, 128], BF16)
            nc.gpsimd.memset(wdum[:], 1.0)
            xdum = dumpool.tile([128, 512], BF16)
            nc.gpsimd.memset(xdum[:], 1.0)
            psdum = psepool.tile([128, 512], F32, name="psdum", tag="pse")
            for i in range(NWARM):
                nc.tensor.matmul(psdum[:], wdum[:], xdum[:],
                                 start=(i == 0), stop=(i == NWARM - 1))

            # --- centroid prep: one DMA for all 10 d-tiles, then
            # cmod = 1 - 2c (bf16). Pad rows are 0.5 -> cmod = 0.
            ct = cpool.tile([128, ND * C], F32)
            nc.gpsimd.dma_start(
                ct[:].rearrange("p (a c) -> p a c", c=C),
                cT.ap().rearrange("(a p) c -> p a c", p=128))
            cmod = cmpool.tile([128, ND * C], BF16)
            nc.scalar.activation(cmod[:], ct[:], AF.Copy, bias=1.0,
                                 scale=-2.0)
            cmods = [cmod[:, di * C:(di + 1) * C] for di in range(ND)]

            # --- input loads; tiles single-assignment (loaded once, no
            # slot reuse) so input DMAs never carry data waits. x tiles
            # span two b-blocks, w tiles two d-tiles. DoubleRow pairs
            # (k-tile t in dim 1) are stacked per-tile: element
            # [p, t, n] is feature 256j + 128t + p.
            xts = {}
            xtails = {}
            wts = {}
            wtails = {}

            def load_x(bb, j):    # per b-block: 128 KB DMAs so the
                # first group's tiles clear their queues fast
                xt = xpool.tile([128, 2, 512], FP8, name=f"xt_{bb}_{j}",
                                tag="xt")
                nc.sync.dma_start(
                    xt[:],
                    xT[256 * j:256 * (j + 1),
                       bb * 512:(bb + 1) * 512].rearrange(
                           "(t p) b -> p t b", p=128))
                xts[bb, j] = xt

            def load_xtail(bp):
                xt = xtpool.tile([FTAIL, 1024], FP8, name=f"xtl_{bp}",
                                 tag="xtl")
                nc.sync.dma_start(
                    xt[:], xT[NJ * 256:F, bp * 1024:(bp + 1) * 1024])
                xtails[bp] = xt

            def load_w(dp, j):    # dp = d-tile pair index (0..4)
                wid = min(256, DP - dp * 256)
                wt = wpool.tile([128, 2, wid], FP8, name=f"wt_{dp}_{j}",
                                tag="wt")
                nc.gpsimd.dma_start(
                    wt[:],
                    wT[256 * j:256 * (j + 1),
                       dp * 256:dp * 256 + wid].rearrange(
                           "(t p) d -> p t d", p=128))
                wts[dp, j] = wt

            def load_wtail(dp):
                wid = min(256, DP - dp * 256)
                wt = wtpool.tile([FTAIL, wid], FP8, name=f"wtl_{dp}",
                                 tag="wtl")
                nc.gpsimd.dma_start(
                    wt[:], wT[NJ * 256:F, dp * 256:dp * 256 + wid])
                wtails[dp] = wt

            # first wave: everything the first two d-tiles (dp0) of
            # b-group 0 need, then the rest in streaming order
            for bb in range(4):
                for j in range(NJ):
                    load_x(bb, j)
            load_xtail(0)
            load_xtail(1)
            for j in range(NJ):
                load_w(0, j)
            load_wtail(0)
            for i in range(1, 5):
                for j in range(NJ):
                    load_x(3 + i, j)
                    load_w(i, j)
                if i >= 3:
                    load_xtail(i - 1)
                load_wtail(i)

            def xop(bb, j):
                return xts[bb, j][:]

            def xtailop(bb):
                return xtails[bb // 2][:, (bb % 2) * 512:(bb % 2 + 1) * 512]

            def wop(di, j):
                return wts[di // 2, j][:, :,
                                       (di % 2) * 128:(di % 2 + 1) * 128]

            def wtailop(di):
                return wtails[di // 2][:, (di % 2) * 128:(di % 2 + 1) * 128]

            # --- main compute: two b-groups of 4 blocks.
            for bg in range(2):
                bbs = list(range(bg * 4, bg * 4 + 4))
                ps2 = ps2pool.tile([128, 512], F32, name=f"ps2_{bg}",
                                   tag="ps2")
                psum2 = {bb: ps2[32 * (bb % 4):32 * (bb % 4) + C, :]
                         for bb in bbs}
                pending = []
                for di in range(ND):
                    pses = {}
                    for bb in bbs:
                        pses[bb] = psepool.tile([128, 512], F32,
                                                name=f"pse_{di % 2}_{bb}",
                                                tag="pse")
                    for j in range(NJ):
                        for bb in bbs:
                            nc.tensor.matmul(pses[bb][:], wop(di, j),
                                             xop(bb, j),
                                             start=(j == 0), stop=False,
                                             perf_mode=DR)
                    for bb in bbs:
                        nc.tensor.matmul(pses[bb][:], wtailop(di),
                                         xtailop(bb),
                                         start=False, stop=True)
                    ets = {}
                    for bb in bbs:
                        # e' = (hv > 0) - 0.5 in {-1/2, +1/2}; the last
                        # d-tile binarizes in halves so its hamming
                        # matmuls overlap the binarize (no encode work
                        # left to hide the chain behind)
                        et = epool.tile([128, 512], BF16,
                                        name=f"et_{di % 2}_{bb}", tag="et")
                        if di == ND - 1:
                            for h in range(2):
                                sl = slice(h * 256, (h + 1) * 256)
                                nc.vector.tensor_scalar(
                                    et[:, sl], pses[bb][:, sl], 0.0, 0.5,
                                    op0=OP.is_gt, op1=OP.subtract)
                        else:
                            nc.vector.tensor_scalar(et[:], pses[bb][:],
                                                    0.0, 0.5,
                                                    op0=OP.is_gt,
                                                    op1=OP.subtract)
                        ets[bb] = et
                    for pdi, pbb, pet in pending:
                        nc.tensor.matmul(psum2[pbb], cmods[pdi],
                                         pet[:], start=(pdi == 0),
                                         stop=(pdi == ND - 1),
                                         tile_position=(0, 32 * (pbb % 4)))
                    pending = [(di, bb, ets[bb]) for bb in bbs]
                for pdi, pbb, pet in pending:
                    for h in range(2):
                        sl = slice(h * 256, (h + 1) * 256)
                        nc.tensor.matmul(psum2[pbb][:, sl], cmods[pdi],
                                         pet[:, sl], start=(pdi == 0),
                                         stop=(pdi == ND - 1),
                                         tile_position=(0, 32 * (pbb % 4)))
                    # out = -(psum2 + DREAL/2); alternate engines so the
                    # four epilogues drain in parallel
                    ot = opool.tile([C, 512], F32, name=f"ot_{pbb % 4}",
                                    tag="ot")
                    if pbb % 2 == 0:
                        nc.scalar.activation(ot[:], psum2[pbb], AF.Copy,
                                             bias=-float(DREAL) / 2.0,
                                             scale=-1.0)
                    else:
                        nc.vector.tensor_scalar(ot[:], psum2[pbb],
                                                float(DREAL) / 2.0, -1.0,
                                                op0=OP.add, op1=OP.mult)
                    nc.gpsimd.dma_start(
                        out[:, pbb * 512:(pbb + 1) * 512], ot[:])
    nc.compile()
    _NC_CACHE["nc"] = nc
    return nc


def _prep_in_maps(samples, bhv_matrix, centroids):
    samples = np.ascontiguousarray(samples, dtype=np.float32)
    bhv_matrix = np.ascontiguousarray(bhv_matrix, dtype=np.float32)
    centroids = np.ascontiguousarray(centroids, dtype=np.float32)
    xT = np.ascontiguousarray((samples.T - 0.5).astype(NP_FP8))  # [F, B]
    w8 = bhv_matrix.astype(NP_FP8)
    in_maps = []
    for k in range(NCORES):
        lo_, hi_ = k * DREAL, (k + 1) * DREAL
        wTk = np.zeros((F, DP), dtype=NP_FP8)
        wTk[:, :DREAL] = w8[lo_:hi_, :].T
        cTk = np.full((DP, C), 0.5, dtype=np.float32)
        cTk[:DREAL, :] = centroids[:, lo_:hi_].T
        in_maps.append({"xT": xT, "wT": wTk, "cT": cTk})
    return in_maps


def _run(samples, bhv_matrix, centroids, **spmd_kwargs):
    nc = _build_nc()
    in_maps = _prep_in_maps(samples, bhv_matrix, centroids)
    res = run_bass_kernel_spmd(nc, in_maps, core_ids=list(range(NCORES)),
                               **spmd_kwargs)
    acc = np.zeros((C, B), dtype=np.float32)
    for r in res.results:
        acc += r["out"]
    return np.ascontiguousarray(acc.T), res


def kernel(samples, bhv_matrix, centroids):
    out, _ = _run(samples, bhv_matrix, centroids)
    return out


# revision 11
# speedup vs baseline: 1.6666x; 1.0086x over previous
"""Trainium2 Bass kernel for nn_BaselineMNISTClassifier (vq_codebook).

reference:
    x = samples - 0.5                        # [B, F]
    hv = einsum('bf,df->bd', x, bhv)         # [B, D]
    e = (hv > 0)                             # binary
    ham[b, c] = sum_d |e - centroids[c, d]|  # [B, C]
    return -ham

Identity used on device: with e' = (hv > 0) - 0.5 in {-1/2, +1/2} and
cmod = 1 - 2c in {-1, +1}:  |e - c| = e' * cmod + 1/2, so
    ham[b, c] = sum_d e'[b, d] * cmod[c, d] + D/2
which turns the broadcast Hamming into a second (tiny) matmul over the
same d-tiles.

Sharding: the D axis (10000) splits across 8 cores, 1250 (zero-padded
to 1280) per core. Every core sees the full batch and computes a
partial hamming [C, B]; the partials sum on the host (padded dims
contribute exactly 0: the centroid pad value 0.5 makes cmod = 0 there).

Encode matmul runs in fp8 e4m3 with perf_mode=DoubleRow: 784 features
= 3 DoubleRow pairs (k=2x128) + one plain k=16 fp8 tail per output
tile, vs 7 fp32r matmuls before (~1.6x less PE stream time; measured
rel err 8e-3 vs the 2e-2 gate, validated in numpy against the fp32
reference). x is centered and cast to e4m3 on the host, which also
removes the on-device DVE centering pass. The hamming matmul stays
bf16 (e', cmod exact in bf16) with all four accumulators of a b-group
col-tiled into ONE PSUM bank (tile_position), freeing 7 banks for the
encode accumulation (DoubleRow + col tiling are mutually exclusive on
the XBUS budget, so the hamming matmuls must stay off perf_mode).

Perf structure (per core):
  - 52 warmup matmuls on dummy data release the PE HAM clock gate
    (1.2 -> 2.4 GHz) while the inputs stream in
  - input tiles are single-assignment (no slot reuse), so input DMAs
    never carry data-dependency waits; x triggers issue from SP, w and
    centroid/output triggers from GpSimd
  - j-outer / bb-inner matmul order: 4 consecutive matmuls share the
    stationary weights, hiding the DoubleRow LDWEIGHTS
  - hamming matmuls are emitted one d-tile late so the PE never waits
    on the DVE binarize; the epilogue alternates Scalar/DVE and each
    output block DMAs out as soon as its accumulation closes

Toolchain notes: built on bacc.Bacc (its compile() legalizes the
1-sync-wait-per-instruction hardware limit via event semaphores);
output DMAs go through nc.gpsimd because SP DMA_DIRECT2D triggers
only take a single wait.
"""

import sys

sys.path.insert(0, "/opt/trn_rl_repo")

import ml_dtypes
import numpy as np

import concourse.bacc as bacc
import concourse.bass as bass
import concourse.mybir as mybir
import concourse.tile as tile
from concourse.bass_utils import run_bass_kernel_spmd

B = 4096
F = 784
D = 10000
C = 10
NCORES = 8
DREAL = D // NCORES          # 1250 real dims per core
DP = 1280                    # padded to 10 d-tiles of 128
ND = DP // 128               # 10
NB = B // 512                # 8 b-blocks of 512
NJ = 3                       # DoubleRow f-pairs (k=256 each)
FTAIL = F - NJ * 256         # 16 leftover features
NWARM = 8                    # PE warmup matmuls (sized to cover the
                             # input-DMA wait; the HAM clock ramp
                             # finishes during the first real d-tiles)

F32 = mybir.dt.float32
FP8 = mybir.dt.float8e4
BF16 = mybir.dt.bfloat16
OP = mybir.AluOpType
AF = mybir.ActivationFunctionType
DR = mybir.MatmulPerfMode.DoubleRow

NP_FP8 = mybir.dt.np(FP8)

_NC_CACHE = {}


def _build_nc():
    if "nc" in _NC_CACHE:
        return _NC_CACHE["nc"]
    nc = bacc.Bacc("TRN2", debug=False, target_bir_lowering=False)
    xT = nc.dram_tensor("xT", [F, B], FP8, kind="ExternalInput")
    wT = nc.dram_tensor("wT", [F, DP], FP8, kind="ExternalInput")
    cT = nc.dram_tensor("cT", [DP, C], F32, kind="ExternalInput")
    out = nc.dram_tensor("out", [C, B], F32, kind="ExternalOutput")

    with tile.TileContext(nc) as tc:
        with (
            tc.tile_pool(name="dum", bufs=2) as dumpool,
            tc.tile_pool(name="xp", bufs=NB * NJ) as xpool,
            tc.tile_pool(name="xtp", bufs=NB // 2) as xtpool,
            tc.tile_pool(name="wp", bufs=(ND + 1) // 2 * NJ) as wpool,
            tc.tile_pool(name="wtp", bufs=(ND + 1) // 2) as wtpool,
            tc.tile_pool(name="cp", bufs=1) as cpool,
            tc.tile_pool(name="cmp", bufs=1) as cmpool,
            tc.tile_pool(name="ep", bufs=8) as epool,
            tc.tile_pool(name="op", bufs=4) as opool,
            tc.tile_pool(name="pse", bufs=7, space="PSUM") as psepool,
            tc.tile_pool(name="ps2", bufs=1, space="PSUM") as ps2pool,
        ):
            # --- dummy tiles for PE warmup; memset on vector (idle
            # early) so the warmup matmuls can start as soon as the
            # sequencer preamble ends
            wdum = dumpool.tile([128, 128], BF16)
            nc.vector.memset(wdum[:], 1.0)
            xdum = dumpool.tile([128, 512], BF16)
            nc.vector.memset(xdum[:], 1.0)

            # --- centroid prep: one DMA for all 10 d-tiles, then
            # cmod = 1 - 2c (bf16). Pad rows are 0.5 -> cmod = 0.
            ct = cpool.tile([128, ND * C], F32)
            nc.gpsimd.dma_start(
                ct[:].rearrange("p (a c) -> p a c", c=C),
                cT.ap().rearrange("(a p) c -> p a c", p=128))
            cmod = cmpool.tile([128, ND * C], BF16)
            nc.scalar.activation(cmod[:], ct[:], AF.Copy, bias=1.0,
                                 scale=-2.0)
            cmods = [cmod[:, di * C:(di + 1) * C] for di in range(ND)]

            # --- input loads; tiles single-assignment (loaded once, no
            # slot reuse) so input DMAs never carry data waits. x tiles
            # span two b-blocks, w tiles two d-tiles. DoubleRow pairs
            # (k-tile t in dim 1) are stacked per-tile: element
            # [p, t, n] is feature 256j + 128t + p.
            xts = {}
            xtails = {}
            wts = {}
            wtails = {}

            def load_x(bb, j, eng=None):   # per b-block: 128 KB DMAs so
                # the first group's tiles clear their queues fast
                xt = xpool.tile([128, 2, 512], FP8, name=f"xt_{bb}_{j}",
                                tag="xt")
                (eng or nc.sync).dma_start(
                    xt[:],
                    xT[256 * j:256 * (j + 1),
                       bb * 512:(bb + 1) * 512].rearrange(
                           "(t p) b -> p t b", p=128))
                xts[bb, j] = xt

            def load_xtail(bp, eng=None):
                xt = xtpool.tile([FTAIL, 1024], FP8, name=f"xtl_{bp}",
                                 tag="xtl")
                (eng or nc.sync).dma_start(
                    xt[:], xT[NJ * 256:F, bp * 1024:(bp + 1) * 1024])
                xtails[bp] = xt

            def load_w(dp, j):    # dp = d-tile pair index (0..4)
                wid = min(256, DP - dp * 256)
                wt = wpool.tile([128, 2, wid], FP8, name=f"wt_{dp}_{j}",
                                tag="wt")
                nc.gpsimd.dma_start(
                    wt[:],
                    wT[256 * j:256 * (j + 1),
                       dp * 256:dp * 256 + wid].rearrange(
                           "(t p) d -> p t d", p=128))
                wts[dp, j] = wt

            def load_wtail(dp):
                wid = min(256, DP - dp * 256)
                wt = wtpool.tile([FTAIL, wid], FP8, name=f"wtl_{dp}",
                                 tag="wtl")
                nc.gpsimd.dma_start(
                    wt[:], wT[NJ * 256:F, dp * 256:dp * 256 + wid])
                wtails[dp] = wt

            # First wave: everything b-group 0's first d-tiles need,
            # issue spread across engines (a DMA trigger costs ~0.6us
            # of issue time on its engine, so a single engine can't
            # get the first group's 20 tiles in flight fast enough).
            # tensor's triggers run before its warmup matmuls.
            for bb in range(4):
                load_x(bb, 0, nc.sync)       # j0: needed at t=0
            for bb in range(4):
                load_x(bb, 1, nc.scalar)     # j1: needed at +1us
            for bb in range(4):
                load_x(bb, 2, nc.sync)       # j2: needed at +2us
            for j in range(NJ):
                load_w(0, j)
            load_xtail(0)                    # tails: needed at +3us
            load_xtail(1)
            load_wtail(0)

            # --- PE warmup: release the HAM clock gate while the
            # first-wave inputs stream in.
            psdum = psepool.tile([128, 512], F32, name="psdum", tag="pse")
            for i in range(NWARM):
                nc.tensor.matmul(psdum[:], wdum[:], xdum[:],
                                 start=(i == 0), stop=(i == NWARM - 1))

            # rest of the inputs in streaming order
            for i in range(1, 5):
                for j in range(NJ):
                    load_x(3 + i, j, nc.sync)
                    load_w(i, j)
                if i >= 3:
                    load_xtail(i - 1, nc.sync)
                load_wtail(i)

            def xop(bb, j):
                return xts[bb, j][:]

            def xtailop(bb):
                return xtails[bb // 2][:, (bb % 2) * 512:(bb % 2 + 1) * 512]

            def wop(di, j):
                return wts[di // 2, j][:, :,
                                       (di % 2) * 128:(di % 2 + 1) * 128]

            def wtailop(di):
                return wtails[di // 2][:, (di % 2) * 128:(di % 2 + 1) * 128]

            # --- main compute: two b-groups of 4 blocks.
            for bg in range(2):
                bbs = list(range(bg * 4, bg * 4 + 4))
                ps2 = ps2pool.tile([128, 512], F32, name=f"ps2_{bg}",
                                   tag="ps2")
                psum2 = {bb: ps2[32 * (bb % 4):32 * (bb % 4) + C, :]
                         for bb in bbs}
                pending = []
                for di in range(ND):
                    pses = {}
                    for bb in bbs:
                        pses[bb] = psepool.tile([128, 512], F32,
                                                name=f"pse_{di % 2}_{bb}",
                                                tag="pse")
                    for j in range(NJ):
                        for bb in bbs:
                            nc.tensor.matmul(pses[bb][:], wop(di, j),
                                             xop(bb, j),
                                             start=(j == 0), stop=False,
                                             perf_mode=DR)
                    for bb in bbs:
                        nc.tensor.matmul(pses[bb][:], wtailop(di),
                                         xtailop(bb),
                                         start=False, stop=True)
                    ets = {}
                    for bb in bbs:
                        # e' = (hv > 0) - 0.5 in {-1/2, +1/2}; the last
                        # d-tile binarizes in halves so its hamming
                        # matmuls overlap the binarize (no encode work
                        # left to hide the chain behind)
                        et = epool.tile([128, 512], BF16,
                                        name=f"et_{di % 2}_{bb}", tag="et")
                        if di == ND - 1:
                            for h in range(2):
                                sl = slice(h * 256, (h + 1) * 256)
                                nc.vector.tensor_scalar(
                                    et[:, sl], pses[bb][:, sl], 0.0, 0.5,
                                    op0=OP.is_gt, op1=OP.subtract)
                        else:
                            nc.vector.tensor_scalar(et[:], pses[bb][:],
                                                    0.0, 0.5,
                                                    op0=OP.is_gt,
                                                    op1=OP.subtract)
                        ets[bb] = et
                    for pdi, pbb, pet in pending:
                        nc.tensor.matmul(psum2[pbb], cmods[pdi],
                                         pet[:], start=(pdi == 0),
                                         stop=(pdi == ND - 1),
                                         tile_position=(0, 32 * (pbb % 4)))
                    pending = [(di, bb, ets[bb]) for bb in bbs]
                for pdi, pbb, pet in pending:
                    for h in range(2):
                        sl = slice(h * 256, (h + 1) * 256)
                        nc.tensor.matmul(psum2[pbb][:, sl], cmods[pdi],
                                         pet[:, sl], start=(pdi == 0),
                                         stop=(pdi == ND - 1),
                                         tile_position=(0, 32 * (pbb % 4)))
                    # out = -(psum2 + DREAL/2); alternate engines so the
                    # four epilogues drain in parallel
                    ot = opool.tile([C, 512], F32, name=f"ot_{pbb % 4}",
                                    tag="ot")
                    if pbb % 2 == 0:
                        nc.scalar.activation(ot[:], psum2[pbb], AF.Copy,
                                             bias=-float(DREAL) / 2.0,
                                             scale=-1.0)
                    else:
                        nc.vector.tensor_scalar(ot[:], psum2[pbb],
                                                float(DREAL) / 2.0, -1.0,
                                                op0=OP.add, op1=OP.mult)
                    nc.gpsimd.dma_start(
                        out[:, pbb * 512:(pbb + 1) * 512], ot[:])
    nc.compile()
    _NC_CACHE["nc"] = nc
    return nc


def _prep_in_maps(samples, bhv_matrix, centroids):
    samples = np.ascontiguousarray(samples, dtype=np.float32)
    bhv_matrix = np.ascontiguousarray(bhv_matrix, dtype=np.float32)
    centroids = np.ascontiguousarray(centroids, dtype=np.float32)
    xT = np.ascontiguousarray((samples.T - 0.5).astype(NP_FP8))  # [F, B]
    w8 = bhv_matrix.astype(NP_FP8)
    in_maps = []
    for k in range(NCORES):
        lo_, hi_ = k * DREAL, (k + 1) * DREAL
        wTk = np.zeros((F, DP), dtype=NP_FP8)
        wTk[:, :DREAL] = w8[lo_:hi_, :].T
        cTk = np.full((DP, C), 0.5, dtype=np.float32)
        cTk[:DREAL, :] = centroids[:, lo_:hi_].T
        in_maps.append({"xT": xT, "wT": wTk, "cT": cTk})
    return in_maps


def _run(samples, bhv_matrix, centroids, **spmd_kwargs):
    nc = _build_nc()
    in_maps = _prep_in_maps(samples, bhv_matrix, centroids)
    res = run_bass_kernel_spmd(nc, in_maps, core_ids=list(range(NCORES)),
                               **spmd_kwargs)
    acc = np.zeros((C, B), dtype=np.float32)
    for r in res.results:
        acc += r["out"]
    return np.ascontiguousarray(acc.T), res


def kernel(samples, bhv_matrix, centroids):
    out, _ = _run(samples, bhv_matrix, centroids)
    return out
